# revision 56
# baseline (speedup 1.0000x reference)
"""Causal self-attention (GQA + RoPE) Trainium2 kernel, 8-way sharded.

Sharding: DP=4 over batch x TP=2 over kv-head groups (2 kv heads + their
8 q heads per group).  Each core computes its batch's qkv projection for
its head group, causal attention, and a partial c_proj (columns of
w_proj for its head group).  Host sums the two partial c_proj outputs
per batch.

Everything on-chip runs transposed ([feature, token] layout) so matmuls
contract along partitions; host transposes inputs/outputs.

Pipeline: the attention inner loop is ACT-bound (one exp per QK tile),
so the q/k projection + RoPE work for head h+1 is interleaved into the
PE stream of head h's attention, keeping the PE busy while ACT churns
through exps.

RoPE: w_attn q/k rows are permuted per-head to [even dims; odd dims] so
rotation pairs land at partition f and f+64 of the qkv psum tile:
  P  = ps * [c; c] (SBUF),  P2 = ps * [s; s] (PSUM)
  out[0:64]   = P[0:64]  - P2[64:128]
  out[64:128] = P2[0:64] + P[64:128]
(each combine reads one SBUF + one PSUM operand, which may sit at
different base partitions; two SBUF operands may not).

Softmax: att^T tiles ([k, q] layout) are exp'd on ACT without
max-subtraction (logits are O(6), fp32-safe).  Denominators: groups of
4 e-tiles are tree-summed on DVE and hit with one ones-column matmul
per group (deferred into the next group's PE stream); the per-q
reciprocal is broadcast down partitions with a f32r outer-product
matmul, also deferred one q-tile.
"""

import math

import numpy as np
import ml_dtypes

import concourse.bass as bass
import concourse.bass_isa as bass_isa
import concourse.mybir as mybir
import concourse.tile as tile
from concourse import bacc
from concourse.bass_utils import run_bass_kernel_spmd

ALU = mybir.AluOpType
AF = mybir.ActivationFunctionType
F32 = mybir.dt.float32
F32R = mybir.dt.float32r
BF16 = mybir.dt.bfloat16
BF = ml_dtypes.bfloat16

# problem shape (hardcoded per contest rules)
B, T, C = 4, 2048, 2048
N_HEAD, N_KV_HEAD, HD = 16, 4, 128
ROPE_THETA = 10000.0

TP = 2            # head-group shards
DP = 4            # batch shards
HQ = N_HEAD // TP         # 8 q heads per core
HKV = N_KV_HEAD // TP     # 2 kv heads per core
NREP = N_HEAD // N_KV_HEAD  # 4
QK_ROWS = (HQ + HKV) * HD   # 1280
KC = C // 128     # 16 contraction tiles
NQ = T // 512     # 4 token strips
MQK = QK_ROWS // 128  # 10 feature tiles (8 q heads + 2 kv heads)
FM = C // 128     # 16 output feature tiles
SCALE = 1.0 / math.sqrt(HD)

N_CORES = 8

_NC = None        # cached compiled Bass module
LAST_RUN = None   # BassKernelResults of the most recent kernel() call


def build_nc(dbg=False, tag=None, cfg=None):
    cfg = {**dict(look=1, pop_mode=3, ygran=2), **(cfg or {})}
    nc = bacc.Bacc(None, target_bir_lowering=False, debug=False)

    xT = nc.declare_dram_parameter("xT", [C, T], BF16, isOutput=False)
    wqk3 = nc.declare_dram_parameter("wqk3", [MQK, 128, C], BF16, isOutput=False)
    wv3 = nc.declare_dram_parameter("wv3", [128, KC * HKV * HD], BF16, isOutput=False)
    wp5 = nc.declare_dram_parameter("wp5", [FM, 128, HQ, 128], BF16, isOutput=False)
    trigf = nc.declare_dram_parameter("trigf", [128, T], BF16, isOutput=False)  # [c;c]
    trigw = nc.declare_dram_parameter("trigw", [128, T], BF16, isOutput=False)  # [s;s]
    maskd = nc.declare_dram_parameter("maskd", [4, 128, 512], BF16, isOutput=False)
    outT = nc.declare_dram_parameter("outT", [C, T], F32, isOutput=True)
    if dbg:
        dbg_q = nc.declare_dram_parameter("dbg_q", [128, T], BF16, isOutput=True)
        dbg_k = nc.declare_dram_parameter("dbg_k", [128, T], BF16, isOutput=True)
        dbg_v = nc.declare_dram_parameter(
            "dbg_v", [128, T // 128, HKV * HD], BF16, isOutput=True
        )
        dbg_y = nc.declare_dram_parameter("dbg_y", [128, HQ, T], BF16, isOutput=True)

    with tile.TileContext(nc) as tc:
        with (
            tc.tile_pool(name="const", bufs=1) as const,
            tc.tile_pool(name="persist", bufs=1) as persist,
            tc.tile_pool(name="eb", bufs=8) as eb,
            tc.tile_pool(name="gag", bufs=2) as gag,
            tc.tile_pool(name="smp", bufs=2) as smp,
            tc.tile_pool(name="srp", bufs=2) as srp,
            tc.tile_pool(name="wmear", bufs=1) as wm_early,
            tc.tile_pool(name="psS", bufs=3, space="PSUM") as psS,
            tc.tile_pool(name="psY", bufs=2, space="PSUM") as psY,
        ):
            trigf_sb = const.tile([128, T], BF16, name="trigf")
            trigw_sb = const.tile([128, T], BF16, name="trigw")
            mask_sb = const.tile([128, 4, 512], BF16, name="mask")

            qrot = [persist.tile([128, T], BF16, name=f"qrot{h}") for h in range(HQ)]
            krot = [persist.tile([128, T], BF16, name=f"krot{h}") for h in range(HKV)]
            v_sb = persist.tile([128, T // 128, HKV * HD], BF16, name="vtok")
            yt = persist.tile([128, HQ, T], BF16, name="yt")

            state = {"pending": None}

            def finalize(h, qj, ps_y, ssum):
                if tag:
                    tag(nc, f"finalize h{h} qj{qj}")
                nc.vector.reciprocal(ssum[:], ssum[:])
                nc.vector.tensor_tensor(
                    yt[:, h, bass.ts(qj, 512)], ps_y[:], ssum[:], ALU.mult
                )

            def stage_a(h, qj, kt, pss_x=None):
                """QK matmul + exp (+ causal mask for diagonal tiles)."""
                kvh = h // NREP
                d = kt - 4 * qj
                lo = 128 * d if d > 0 else 0
                qlo = qj * 512 + lo
                if pss_x is not None and kt % 4 == 3:
                    ps_s = pss_x.tile([128, 512], F32, name="pssx")
                else:
                    ps_s = psS.tile([128, 512], F32, name="pss")
                if tag:
                    tag(nc, f"QK h{h} qj{qj} kt{kt}")
                nc.tensor.matmul(
                    ps_s[:, lo:512],
                    krot[kvh][:, kt * 128 : (kt + 1) * 128],
                    qrot[h][:, qlo : (qj + 1) * 512],
                    start=True,
                    stop=True,
                )
                e = eb.tile([128, 512], BF16, name="e")
                nc.scalar.activation(
                    e[:, lo:512], ps_s[:, lo:512], AF.Exp, scale=SCALE
                )
                if d >= 0:
                    nc.vector.tensor_tensor(
                        e[:, lo:512], e[:, lo:512],
                        mask_sb[:, d, lo:512], ALU.mult,
                    )
                return e

            def stage_b(c, h, qj, kt, e):
                """AV matmul + strip-sum accumulation for tile kt.

                On the strip's last tile, issues the GPSIMD
                partition_all_reduce and returns the ssum tile."""
                kvh = h // NREP
                nkt = 4 * qj + 4
                d = kt - 4 * qj
                lo = 128 * d if d > 0 else 0
                if kt == 0:
                    c["ps_y"] = psY.tile([128, 512], F32, name="psy")
                s = c["s"]
                tree = c["tree"]
                if tag:
                    tag(nc, f"AV h{h} qj{qj} kt{kt}")
                nc.tensor.matmul(
                    c["ps_y"][:, lo:512],
                    v_sb[:, kt, kvh * HD : (kvh + 1) * HD],
                    e[:, lo:512],
                    start=(kt == 0),
                    stop=(kt == nkt - 1),
                )
                if d >= 0:
                    if s is None:
                        # qj == 0, d == 0: seed the strip sum
                        s = smp.tile([128, 512], BF16, name="s")
                        c["s"] = s
                        nc.vector.tensor_copy(s[:], e[:])
                    else:
                        nc.vector.tensor_tensor(
                            s[:, lo:512], s[:, lo:512], e[:, lo:512], ALU.add
                        )
                else:
                    # full groups: tree-sum 4 e-tiles on DVE, then merge
                    # into the strip sum.
                    ph = kt % 4
                    if ph == 0:
                        tree["g0"] = e
                    elif ph == 1:
                        ga = gag.tile([128, 512], BF16, name="ga")
                        nc.vector.tensor_tensor(ga[:], tree["g0"][:], e[:], ALU.add)
                        tree["ga"] = ga
                    elif ph == 2:
                        tree["g2"] = e
                    else:
                        if s is None:
                            s = smp.tile([128, 512], BF16, name="s")
                            c["s"] = s
                            gs = s
                        else:
                            gs = gag.tile([128, 512], BF16, name="gs")
                        nc.vector.tensor_tensor(gs[:], tree["g2"][:], e[:], ALU.add)
                        nc.vector.tensor_tensor(gs[:], gs[:], tree["ga"][:], ALU.add)
                        if gs is not s:
                            nc.vector.tensor_tensor(s[:], s[:], gs[:], ALU.add)
                if kt == nkt - 1:
                    ssum = srp.tile([128, 512], F32, name="ssum")
                    nc.gpsimd.partition_all_reduce(
                        ssum[:], s[:], 128, bass_isa.ReduceOp.add
                    )
                    return ssum
                return None

            def emit_strip(h, qj, pop, pss_x=None):
                """One attention strip, QK/exp one tile ahead of AV so the
                AV matmul never waits on ACT's exp latency."""
                c = {"s": None, "tree": {}, "ps_y": None}
                nkt = 4 * qj + 4
                e_prev = stage_a(h, qj, 0, pss_x)
                for kt in range(1, nkt):
                    e_cur = stage_a(h, qj, kt, pss_x)
                    pop(h, qj, kt - 1)
                    stage_b(c, h, qj, kt - 1, e_prev)
                    e_prev = e_cur
                ssum = stage_b(c, h, qj, nkt - 1, e_prev)
                pop(h, qj, nkt - 1)
                return c["ps_y"], ssum

            # ======== projection machinery (lives through heads 0..6) ========
            with (
                tc.tile_pool(name="xa", bufs=1) as xa,
                tc.tile_pool(name="wm", bufs=3) as wm,
                tc.tile_pool(name="ta", bufs=1) as ta,
                tc.tile_pool(name="psA", bufs=2, space="PSUM") as psA,
                tc.tile_pool(name="psP2", bufs=1, space="PSUM") as psP2,
            ):
                xs = xa.tile([128, KC, T], BF16, name="xs")

                def load_wm(m, split=1):
                    w = wm.tile([128, KC, 128], BF16, name="wm")
                    wsrc = wqk3[m, :, :].rearrange("p (kc c) -> p kc c", kc=KC)
                    step = KC // split
                    chunks = []
                    for i in range(split):
                        chunks.append(
                            lambda i=i: nc.sync.dma_start(
                                w[:, i * step : (i + 1) * step, :],
                                wsrc[:, i * step : (i + 1) * step, :],
                            )
                        )
                    if split == 1:
                        chunks[0]()
                        return w
                    return w, chunks

                def rope_thunks(m, n, ps):
                    """The four RoPE ops for one (feature tile, strip) pair,
                    as emission thunks (must be called in list order).  The
                    sine product goes to a PSUM scratch tile so `ps` (the
                    projection accumulator) is released after the two
                    products, and so each combine reads one SBUF + one PSUM
                    operand at different base partitions."""
                    dst = qrot[m] if m < HQ else krot[m - HQ]
                    nsl = bass.ts(n, 512)
                    box = {}

                    def t0():
                        if tag:
                            tag(nc, f"rope m{m} n{n}")
                        box["pt"] = ta.tile([128, 512], F32, name="pt")
                        nc.vector.tensor_tensor(
                            box["pt"][:], ps[:], trigf_sb[:, nsl], ALU.mult
                        )

                    def t1():
                        box["p2"] = psP2.tile([128, 512], F32, name="p2")
                        nc.vector.tensor_tensor(
                            box["p2"][:], ps[:], trigw_sb[:, nsl], ALU.mult
                        )

                    def t2():
                        nc.vector.tensor_tensor(
                            dst[0:64, nsl], box["pt"][0:64, :],
                            box["p2"][64:128, :], ALU.subtract,
                        )

                    def t3():
                        nc.vector.tensor_tensor(
                            dst[64:128, nsl], box["p2"][0:64, :],
                            box["pt"][64:128, :], ALU.add,
                        )

                    return [t0, t1, t2, t3]

                def a_stream(m, pool):
                    if tag:
                        tag(nc, f"a_stream m{m} load_wm")
                    w = load_wm(m)
                    yield
                    for n in range(NQ):
                        nsl = bass.ts(n, 512)
                        if tag:
                            tag(nc, f"a_stream m{m} n{n} mm")
                        ps = pool.tile([128, 512], F32, name="psA")
                        for kc in range(KC):
                            nc.tensor.matmul(
                                ps[:],
                                w[:, kc, :],
                                xs[:, kc, nsl],
                                start=(kc == 0),
                                stop=(kc == KC - 1),
                            )
                            if kc % cfg["ygran"] == cfg["ygran"] - 1:
                                yield
                        for t in rope_thunks(m, n, ps):
                            t()
                            yield

                # ---- A0: v projection + k heads + q head 0 (pure PE phase) ----
                with tc.tile_pool(name="wvp", bufs=1) as wvp:
                    wv_sb = wvp.tile([128, KC, HKV * HD], BF16, name="wv")
                    wvsrc = wv3.rearrange("p (kc c) -> p kc c", kc=KC)
                    xTr = xT.rearrange("(kc p) t -> p kc t", p=128)
                    # all loads issued up front, ordered by first use so the
                    # DMA engine streams while the PE consumes: x(strip 0)
                    # per-kc with wv/wk/wq interleaved at their first-need
                    # points, trig per strip, strip 1 per-kc (sems fire
                    # progressively), strips 2-3 as single big copies, mask
                    # last.
                    def dma_x(kc, n):
                        nc.sync.dma_start(
                            xs[:, kc, bass.ts(n, 512)], xTr[:, kc, bass.ts(n, 512)]
                        )

                    def dma_wv(i):
                        nc.sync.dma_start(
                            wv_sb[:, 4 * i : 4 * i + 4, :],
                            wvsrc[:, 4 * i : 4 * i + 4, :],
                        )

                    def dma_trig(n):
                        nc.sync.dma_start(
                            trigf_sb[:, bass.ts(n, 512)], trigf[:, bass.ts(n, 512)]
                        )
                        nc.sync.dma_start(
                            trigw_sb[:, bass.ts(n, 512)], trigw[:, bass.ts(n, 512)]
                        )

                    dma_x(0, 0)
                    dma_wv(0)
                    wk0 = load_wm(HQ)
                    dma_x(1, 0)
                    dma_x(2, 0)
                    wk1 = load_wm(HQ + 1)
                    dma_x(3, 0)
                    dma_wv(1)
                    wq0 = load_wm(0)
                    for kc in range(4, 8):
                        dma_x(kc, 0)
                    dma_wv(2)
                    for kc in range(8, 12):
                        dma_x(kc, 0)
                    dma_wv(3)
                    for kc in range(12, KC):
                        dma_x(kc, 0)
                    dma_trig(0)
                    for kc in range(KC):
                        dma_x(kc, 1)
                    dma_trig(1)
                    nc.sync.dma_start(
                        xs[:, :, bass.ts(2, 512)], xTr[:, :, bass.ts(2, 512)]
                    )
                    dma_trig(2)
                    nc.sync.dma_start(
                        xs[:, :, bass.ts(3, 512)], xTr[:, :, bass.ts(3, 512)]
                    )
                    dma_trig(3)
                    nc.sync.dma_start(mask_sb[:], maskd.rearrange("d p q -> p d q"))

                    # per-kc interleave: the PE tracks the x DMA stream (one
                    # kc's worth of v+k+q matmuls per arriving tile) instead
                    # of waiting for a full strip.  k/q matmuls lag the v
                    # matmuls by 2 kc so their weight DMAs have landed.
                    LAG = cfg.get("lag", 2)
                    for n in range(NQ):
                        nsl = bass.ts(n, 512)
                        if tag:
                            tag(nc, f"A0 n{n}")
                        kq = [
                            (HQ, wk0, psY.tile([128, 512], F32, name="psy")),
                            (HQ + 1, wk1, psY.tile([128, 512], F32, name="psy")),
                            (0, wq0, psA.tile([128, 512], F32, name="psA")),
                        ]
                        # two half-passes of 2 token-tiles each, one PSUM
                        # tile per token-tile (independent accumulation
                        # groups must not share a tile); k0/k1 lag the pass-0
                        # v matmuls by LAG kc, q0 rides pass 1, so each
                        # projection starts right as its weight DMA lands
                        # while pass 0 paces the x stream.
                        for pas in range(2):
                            vt = [
                                psS.tile([128, 512], F32, name="pss")
                                for _ in range(2)
                            ]
                            for kcv in range(KC + (LAG if pas == 0 else 0)):
                                if kcv < KC:
                                    for i in range(2):
                                        tt = 4 * n + 2 * pas + i
                                        nc.tensor.matmul(
                                            vt[i][:, 0 : HKV * HD],
                                            xs[:, kcv, tt * 128 : (tt + 1) * 128],
                                            wv_sb[:, kcv, :],
                                            start=(kcv == 0),
                                            stop=(kcv == KC - 1),
                                        )
                                    if pas == 1:
                                        for m, w, ps in kq[2:]:
                                            nc.tensor.matmul(
                                                ps[:],
                                                w[:, kcv, :],
                                                xs[:, kcv, nsl],
                                                start=(kcv == 0),
                                                stop=(kcv == KC - 1),
                                            )
                                if pas == 0:
                                    kc = kcv - LAG
                                    if kc >= 0:
                                        for m, w, ps in kq[:2]:
                                            nc.tensor.matmul(
                                                ps[:],
                                                w[:, kc, :],
                                                xs[:, kc, nsl],
                                                start=(kc == 0),
                                                stop=(kc == KC - 1),
                                            )
                            for i in range(2):
                                tt = 4 * n + 2 * pas + i
                                nc.scalar.copy(
                                    v_sb[:, tt, :], vt[i][:, 0 : HKV * HD]
                                )
                        for m, w, ps in kq:
                            for t in rope_thunks(m, n, ps):
                                t()

                # ---- heads 0..6: attention + next head's projection ----
                # preload the first 4 c_proj weight tiles while the DMA
                # engine is idle (the rest need xs's SBUF, freed after head 6)
                wmca = wm_early.tile([128, 4, HQ, 128], BF16, name="wpcearly")
                for fm in range(4):
                    nc.sync.dma_start(wmca[:, fm, :, :], wp5[fm, :, :, :])
                agens = {}

                def get_agen(hn):
                    if hn not in agens and hn < HQ:
                        agens[hn] = a_stream(hn, psA)
                    return agens.get(hn)

                def pop06(h, qj, kt):
                    g = get_agen(h + 1)
                    if g is None:
                        return
                    next(g, None)
                    pm = cfg["pop_mode"]
                    extra = (
                        (kt < 5 or kt >= 10) if pm == 0
                        else kt >= 4 if pm == 1
                        else True if pm == 2
                        else (kt < 4 * qj) if pm == 3
                        else (kt < 4 * qj and qj < 3) if pm == 5
                        else False
                    )
                    if extra:
                        next(g, None)

                def head_end06(h):
                    g = get_agen(h + 1)
                    if g is not None:
                        for _ in g:
                            pass

                for h in range(HQ - 1):
                    for qj in range(NQ):
                        ps_y, ssum = emit_strip(h, qj, pop06)
                        if state["pending"] is not None:
                            finalize(*state["pending"])
                        state["pending"] = (h, qj, ps_y, ssum)
                    head_end06(h)

            # ---- head 7: attention + output projection interleaved ----
            with (
                tc.tile_pool(name="wpc", bufs=1) as wpc,
                tc.tile_pool(name="obp", bufs=3) as obp,
                tc.tile_pool(name="psO", bufs=2, space="PSUM") as psO,
                tc.tile_pool(name="psX", bufs=1, space="PSUM") as psX,
            ):
                # all 16 c_proj weight tiles resident (loaded once; xs freed
                # the SBUF above); per-fm DMAs so sems fire progressively
                wmc_all = wpc.tile([128, FM - 4, HQ, 128], BF16, name="wpcall")
                for fm in range(4, FM):
                    nc.sync.dma_start(wmc_all[:, fm - 4, :, :], wp5[fm, :, :, :])

                def wmc(fm):
                    return wmca[:, fm, :, :] if fm < 4 else wmc_all[:, fm - 4, :, :]

                def c_stream(n):
                    """Output projection for token strip n (16 feature tiles)."""
                    nsl = bass.ts(n, 512)
                    for fm in range(FM):
                        yield
                        if tag:
                            tag(nc, f"cproj n{n} fm{fm}")
                        last = n == NQ - 1 and fm == FM - 1
                        if last:
                            # final tile: compute/copy/store in column halves
                            # (separate PSUM tiles) so the closing DMA chain
                            # is half as deep
                            for c0 in (0, 256):
                                ps_h = psO.tile([128, 512], F32, name="pso")[:, 0:256]
                                for h2 in range(HQ):
                                    nc.tensor.matmul(
                                        ps_h[:],
                                        wmc(fm)[:, h2, :],
                                        yt[:, h2, n * 512 + c0 : n * 512 + c0 + 256],
                                        start=(h2 == 0),
                                        stop=(h2 == HQ - 1),
                                    )
                                ob = obp.tile([128, 256], F32, name="obh")
                                nc.scalar.copy(ob[:], ps_h[:])
                                nc.sync.dma_start(
                                    outT[
                                        fm * 128 : (fm + 1) * 128,
                                        n * 512 + c0 : n * 512 + c0 + 256,
                                    ],
                                    ob[:],
                                )
                            yield
                            continue
                        ps_o = psO.tile([128, 512], F32, name="pso")
                        for h2 in range(HQ):
                            nc.tensor.matmul(
                                ps_o[:],
                                wmc(fm)[:, h2, :],
                                yt[:, h2, nsl],
                                start=(h2 == 0),
                                stop=(h2 == HQ - 1),
                            )
                            if h2 % 2 == 1:
                                yield
                        ob = obp.tile([128, 512], F32, name="ob")
                        nc.scalar.copy(ob[:], ps_o[:])
                        nc.sync.dma_start(
                            outT[fm * 128 : (fm + 1) * 128, nsl], ob[:]
                        )
                        yield

                cgens = []

                _end = object()

                def pop7(kt):
                    for _ in range(2):
                        while cgens:
                            if next(cgens[0], _end) is _end:
                                cgens.pop(0)
                                continue
                            break

                for qj in range(NQ):
                    ps_y, ssum = emit_strip(
                        HQ - 1, qj, lambda h, q, kt: pop7(kt), pss_x=psX
                    )
                    if state["pending"] is not None:
                        finalize(*state["pending"])
                        state["pending"] = None
                    finalize(HQ - 1, qj, ps_y, ssum)
                    cgens.append(c_stream(qj))
                for g in cgens:
                    for _ in g:
                        pass

            if dbg:
                nc.sync.dma_start(dbg_q[:], qrot[0][:])
                nc.sync.dma_start(dbg_k[:], krot[0][:])
                nc.sync.dma_start(dbg_v[:], v_sb[:])
                nc.sync.dma_start(dbg_y[:], yt[:])

    nc.compile()
    return nc


def _get_nc():
    global _NC
    if _NC is None:
        _NC = build_nc()
    return _NC


def _prep_inputs(x, w_attn, w_proj):
    """Build the 8 per-core input maps from the full-problem arrays."""
    perm = np.concatenate([np.arange(0, HD, 2), np.arange(1, HD, 2)])

    f = np.arange(64, dtype=np.float64)
    inv = ROPE_THETA ** (-2.0 * f / HD)
    ang = inv[:, None] * np.arange(T, dtype=np.float64)[None, :]
    trigc = np.cos(ang).astype(np.float32)
    trigs = np.sin(ang).astype(np.float32)
    trigf = np.ascontiguousarray(np.concatenate([trigc, trigc], axis=0)).astype(BF)
    trigw = np.ascontiguousarray(np.concatenate([trigs, trigs], axis=0)).astype(BF)

    kk = np.arange(128)[None, :, None]
    qq = np.arange(512)[None, None, :]
    dd = np.arange(4)[:, None, None]
    maskd = ((128 * dd + kk) <= qq).astype(BF)

    w_attn = np.asarray(w_attn)
    w_proj = np.asarray(w_proj)
    x = np.asarray(x)

    in_maps = []
    for core in range(N_CORES):
        b, g = core // TP, core % TP
        xTa = np.ascontiguousarray(x[b].T).astype(BF)

        qrows = []
        for h in range(HQ):
            gh = g * HQ + h
            qrows.append(gh * HD + perm)
        for kv in range(HKV):
            gk = g * HKV + kv
            qrows.append(N_HEAD * HD + gk * HD + perm)
        qrows = np.concatenate(qrows)
        wqk = w_attn[qrows].astype(BF)  # [1280, C]
        # wqk3[m, p, kc*128+col] = wqk[m*128+col, kc*128+p]
        wqk3 = np.ascontiguousarray(
            wqk.reshape(MQK, 128, KC, 128).transpose(0, 3, 2, 1).reshape(MQK, 128, C)
        )

        vrows = np.concatenate(
            [
                (N_HEAD + N_KV_HEAD) * HD + (g * HKV + kv) * HD + np.arange(HD)
                for kv in range(HKV)
            ]
        )
        wv = w_attn[vrows].astype(BF)  # [256, C]
        # wv3[p, kc*256+c] = wv[c, kc*128+p]
        wv3 = np.ascontiguousarray(
            wv.reshape(HKV * HD, KC, 128).transpose(2, 1, 0).reshape(128, KC * HKV * HD)
        )

        cols = np.arange(g * HQ * HD, (g + 1) * HQ * HD)
        wpg = w_proj[:, cols].astype(BF)  # [C, 1024], rows = out features
        # wp5[fm, d, h, p] = wpg[fm*128+p, h*128+d]
        wp5 = np.ascontiguousarray(
            wpg.T.reshape(HQ, 128, FM, 128).transpose(2, 1, 0, 3)
        )

        in_maps.append(
            {
                "xT": xTa,
                "wqk3": wqk3,
                "wv3": wv3,
                "wp5": wp5,
                "trigf": trigf,
                "trigw": trigw,
                "maskd": maskd,
            }
        )
    return in_maps


def kernel(x, w_attn, w_proj):
    global LAST_RUN
    nc = _get_nc()
    in_maps = _prep_inputs(x, w_attn, w_proj)
    res = run_bass_kernel_spmd(nc, in_maps, core_ids=list(range(N_CORES)))
    LAST_RUN = res
    out = np.empty((B, T, C), dtype=np.float32)
    for b in range(B):
        acc = res.results[TP * b]["outT"] + res.results[TP * b + 1]["outT"]
        out[b] = acc.T
    return out



# revision 64
# speedup vs baseline: 1.0004x; 1.0004x over previous
"""Causal self-attention (GQA + RoPE) Trainium2 kernel, 8-way sharded.

Sharding: DP=4 over batch x TP=2 over kv-head groups (2 kv heads + their
8 q heads per group).  Each core computes its batch's qkv projection for
its head group, causal attention, and a partial c_proj (columns of
w_proj for its head group).  Host sums the two partial c_proj outputs
per batch.

Everything on-chip runs transposed ([feature, token] layout) so matmuls
contract along partitions; host transposes inputs/outputs.

Pipeline: the attention inner loop is ACT-bound (one exp per QK tile),
so the q/k projection + RoPE work for head h+1 is interleaved into the
PE stream of head h's attention, keeping the PE busy while ACT churns
through exps.

RoPE: w_attn q/k rows are permuted per-head to [even dims; odd dims] so
rotation pairs land at partition f and f+64 of the qkv psum tile:
  P  = ps * [c; c] (SBUF),  P2 = ps * [s; s] (PSUM)
  out[0:64]   = P[0:64]  - P2[64:128]
  out[64:128] = P2[0:64] + P[64:128]
(each combine reads one SBUF + one PSUM operand, which may sit at
different base partitions; two SBUF operands may not).

Softmax: att^T tiles ([k, q] layout) are exp'd on ACT without
max-subtraction (logits are O(6), fp32-safe).  Denominators: groups of
4 e-tiles are tree-summed on DVE and hit with one ones-column matmul
per group (deferred into the next group's PE stream); the per-q
reciprocal is broadcast down partitions with a f32r outer-product
matmul, also deferred one q-tile.
"""

import math

import numpy as np
import ml_dtypes

import concourse.bass as bass
import concourse.bass_isa as bass_isa
import concourse.mybir as mybir
import concourse.tile as tile
from concourse import bacc
from concourse.bass_utils import run_bass_kernel_spmd

ALU = mybir.AluOpType
AF = mybir.ActivationFunctionType
F32 = mybir.dt.float32
F32R = mybir.dt.float32r
BF16 = mybir.dt.bfloat16
BF = ml_dtypes.bfloat16

# problem shape (hardcoded per contest rules)
B, T, C = 4, 2048, 2048
N_HEAD, N_KV_HEAD, HD = 16, 4, 128
ROPE_THETA = 10000.0

TP = 2            # head-group shards
DP = 4            # batch shards
HQ = N_HEAD // TP         # 8 q heads per core
HKV = N_KV_HEAD // TP     # 2 kv heads per core
NREP = N_HEAD // N_KV_HEAD  # 4
QK_ROWS = (HQ + HKV) * HD   # 1280
KC = C // 128     # 16 contraction tiles
NQ = T // 512     # 4 token strips
MQK = QK_ROWS // 128  # 10 feature tiles (8 q heads + 2 kv heads)
FM = C // 128     # 16 output feature tiles
SCALE = 1.0 / math.sqrt(HD)

N_CORES = 8

_NC = None        # cached compiled Bass module
LAST_RUN = None   # BassKernelResults of the most recent kernel() call


def build_nc(dbg=False, tag=None, cfg=None):
    cfg = {**dict(look=1, pop_mode=3, ygran=2, eb=10), **(cfg or {})}
    nc = bacc.Bacc(None, target_bir_lowering=False, debug=False)

    xT = nc.declare_dram_parameter("xT", [C, T], BF16, isOutput=False)
    wqk3 = nc.declare_dram_parameter("wqk3", [MQK, 128, C], BF16, isOutput=False)
    wv3 = nc.declare_dram_parameter("wv3", [128, KC * HKV * HD], BF16, isOutput=False)
    wp5 = nc.declare_dram_parameter("wp5", [FM, 128, HQ, 128], BF16, isOutput=False)
    trigf = nc.declare_dram_parameter("trigf", [128, T], BF16, isOutput=False)  # [c;c]
    trigw = nc.declare_dram_parameter("trigw", [128, T], BF16, isOutput=False)  # [s;s]
    maskd = nc.declare_dram_parameter("maskd", [4, 128, 512], BF16, isOutput=False)
    outT = nc.declare_dram_parameter("outT", [C, T], F32, isOutput=True)
    if dbg:
        dbg_q = nc.declare_dram_parameter("dbg_q", [128, T], BF16, isOutput=True)
        dbg_k = nc.declare_dram_parameter("dbg_k", [128, T], BF16, isOutput=True)
        dbg_v = nc.declare_dram_parameter(
            "dbg_v", [128, T // 128, HKV * HD], BF16, isOutput=True
        )
        dbg_y = nc.declare_dram_parameter("dbg_y", [128, HQ, T], BF16, isOutput=True)

    with tile.TileContext(nc) as tc:
        with (
            tc.tile_pool(name="const", bufs=1) as const,
            tc.tile_pool(name="persist", bufs=1) as persist,
            tc.tile_pool(name="eb", bufs=cfg.get("eb", 8)) as eb,
            tc.tile_pool(name="gag", bufs=2) as gag,
            tc.tile_pool(name="smp", bufs=2) as smp,
            tc.tile_pool(name="srp", bufs=2) as srp,
            tc.tile_pool(name="wmear", bufs=1) as wm_early,
            tc.tile_pool(name="psS", bufs=3, space="PSUM") as psS,
            tc.tile_pool(name="psY", bufs=2, space="PSUM") as psY,
        ):
            trigf_sb = const.tile([128, T], BF16, name="trigf")
            trigw_sb = const.tile([128, T], BF16, name="trigw")
            mask_sb = const.tile([128, 4, 512], BF16, name="mask")

            qrot = [persist.tile([128, T], BF16, name=f"qrot{h}") for h in range(HQ)]
            krot = [persist.tile([128, T], BF16, name=f"krot{h}") for h in range(HKV)]
            v_sb = persist.tile([128, T // 128, HKV * HD], BF16, name="vtok")
            yt = persist.tile([128, HQ, T], BF16, name="yt")

            state = {"pending": None}

            def finalize(h, qj, ps_y, ssum):
                if tag:
                    tag(nc, f"finalize h{h} qj{qj}")
                nc.vector.reciprocal(ssum[:], ssum[:])
                nc.vector.tensor_tensor(
                    yt[:, h, bass.ts(qj, 512)], ps_y[:], ssum[:], ALU.mult
                )

            def stage_a(h, qj, kt, pss_x=None):
                """QK matmul + exp (+ causal mask for diagonal tiles)."""
                kvh = h // NREP
                d = kt - 4 * qj
                lo = 128 * d if d > 0 else 0
                qlo = qj * 512 + lo
                if pss_x is not None and kt % 4 == 3:
                    ps_s = pss_x.tile([128, 512], F32, name="pssx")
                else:
                    ps_s = psS.tile([128, 512], F32, name="pss")
                if tag:
                    tag(nc, f"QK h{h} qj{qj} kt{kt}")
                nc.tensor.matmul(
                    ps_s[:, lo:512],
                    krot[kvh][:, kt * 128 : (kt + 1) * 128],
                    qrot[h][:, qlo : (qj + 1) * 512],
                    start=True,
                    stop=True,
                )
                e = eb.tile([128, 512], BF16, name="e")
                nc.scalar.activation(
                    e[:, lo:512], ps_s[:, lo:512], AF.Exp, scale=SCALE
                )
                if d >= 0:
                    nc.vector.tensor_tensor(
                        e[:, lo:512], e[:, lo:512],
                        mask_sb[:, d, lo:512], ALU.mult,
                    )
                return e

            def stage_b(c, h, qj, kt, e):
                """AV matmul + strip-sum accumulation for tile kt.

                On the strip's last tile, issues the GPSIMD
                partition_all_reduce and returns the ssum tile."""
                kvh = h // NREP
                nkt = 4 * qj + 4
                d = kt - 4 * qj
                lo = 128 * d if d > 0 else 0
                if kt == 0:
                    c["ps_y"] = psY.tile([128, 512], F32, name="psy")
                s = c["s"]
                tree = c["tree"]
                if tag:
                    tag(nc, f"AV h{h} qj{qj} kt{kt}")
                nc.tensor.matmul(
                    c["ps_y"][:, lo:512],
                    v_sb[:, kt, kvh * HD : (kvh + 1) * HD],
                    e[:, lo:512],
                    start=(kt == 0),
                    stop=(kt == nkt - 1),
                )
                if d >= 0:
                    if s is None:
                        # qj == 0, d == 0: seed the strip sum
                        s = smp.tile([128, 512], BF16, name="s")
                        c["s"] = s
                        nc.vector.tensor_copy(s[:], e[:])
                    else:
                        nc.vector.tensor_tensor(
                            s[:, lo:512], s[:, lo:512], e[:, lo:512], ALU.add
                        )
                else:
                    # full groups: tree-sum 4 e-tiles on DVE, then merge
                    # into the strip sum.
                    ph = kt % 4
                    if ph == 0:
                        tree["g0"] = e
                    elif ph == 1:
                        ga = gag.tile([128, 512], BF16, name="ga")
                        nc.vector.tensor_tensor(ga[:], tree["g0"][:], e[:], ALU.add)
                        tree["ga"] = ga
                    elif ph == 2:
                        tree["g2"] = e
                    else:
                        if s is None:
                            s = smp.tile([128, 512], BF16, name="s")
                            c["s"] = s
                            gs = s
                        else:
                            gs = gag.tile([128, 512], BF16, name="gs")
                        nc.vector.tensor_tensor(gs[:], tree["g2"][:], e[:], ALU.add)
                        nc.vector.tensor_tensor(gs[:], gs[:], tree["ga"][:], ALU.add)
                        if gs is not s:
                            nc.vector.tensor_tensor(s[:], s[:], gs[:], ALU.add)
                if kt == nkt - 1:
                    ssum = srp.tile([128, 512], F32, name="ssum")
                    nc.gpsimd.partition_all_reduce(
                        ssum[:], s[:], 128, bass_isa.ReduceOp.add
                    )
                    return ssum
                return None

            def emit_strip(h, qj, pop, pss_x=None):
                """One attention strip, QK/exp one tile ahead of AV so the
                AV matmul never waits on ACT's exp latency."""
                c = {"s": None, "tree": {}, "ps_y": None}
                nkt = 4 * qj + 4
                e_prev = stage_a(h, qj, 0, pss_x)
                for kt in range(1, nkt):
                    e_cur = stage_a(h, qj, kt, pss_x)
                    pop(h, qj, kt - 1)
                    stage_b(c, h, qj, kt - 1, e_prev)
                    e_prev = e_cur
                ssum = stage_b(c, h, qj, nkt - 1, e_prev)
                pop(h, qj, nkt - 1)
                return c["ps_y"], ssum

            # ======== projection machinery (lives through heads 0..6) ========
            with (
                tc.tile_pool(name="xa", bufs=1) as xa,
                tc.tile_pool(name="wm", bufs=cfg.get("wm", 3)) as wm,
                tc.tile_pool(name="ta", bufs=1) as ta,
                tc.tile_pool(name="psA", bufs=2, space="PSUM") as psA,
                tc.tile_pool(name="psP2", bufs=1, space="PSUM") as psP2,
            ):
                xs = xa.tile([128, KC, T], BF16, name="xs")

                def load_wm(m, split=1):
                    w = wm.tile([128, KC, 128], BF16, name="wm")
                    wsrc = wqk3[m, :, :].rearrange("p (kc c) -> p kc c", kc=KC)
                    step = KC // split
                    chunks = []
                    for i in range(split):
                        chunks.append(
                            lambda i=i: nc.sync.dma_start(
                                w[:, i * step : (i + 1) * step, :],
                                wsrc[:, i * step : (i + 1) * step, :],
                            )
                        )
                    if split == 1:
                        chunks[0]()
                        return w
                    return w, chunks

                def rope_thunks(m, n, ps):
                    """The four RoPE ops for one (feature tile, strip) pair,
                    as emission thunks (must be called in list order).  The
                    sine product goes to a PSUM scratch tile so `ps` (the
                    projection accumulator) is released after the two
                    products, and so each combine reads one SBUF + one PSUM
                    operand at different base partitions."""
                    dst = qrot[m] if m < HQ else krot[m - HQ]
                    nsl = bass.ts(n, 512)
                    box = {}

                    def t0():
                        if tag:
                            tag(nc, f"rope m{m} n{n}")
                        box["pt"] = ta.tile([128, 512], F32, name="pt")
                        nc.vector.tensor_tensor(
                            box["pt"][:], ps[:], trigf_sb[:, nsl], ALU.mult
                        )

                    def t1():
                        box["p2"] = psP2.tile([128, 512], F32, name="p2")
                        nc.vector.tensor_tensor(
                            box["p2"][:], ps[:], trigw_sb[:, nsl], ALU.mult
                        )

                    def t2():
                        nc.vector.tensor_tensor(
                            dst[0:64, nsl], box["pt"][0:64, :],
                            box["p2"][64:128, :], ALU.subtract,
                        )

                    def t3():
                        nc.vector.tensor_tensor(
                            dst[64:128, nsl], box["p2"][0:64, :],
                            box["pt"][64:128, :], ALU.add,
                        )

                    return [t0, t1, t2, t3]

                def a_stream(m, pool):
                    if tag:
                        tag(nc, f"a_stream m{m} load_wm")
                    w = load_wm(m)
                    yield
                    for n in range(NQ):
                        nsl = bass.ts(n, 512)
                        if tag:
                            tag(nc, f"a_stream m{m} n{n} mm")
                        ps = pool.tile([128, 512], F32, name="psA")
                        for kc in range(KC):
                            nc.tensor.matmul(
                                ps[:],
                                w[:, kc, :],
                                xs[:, kc, nsl],
                                start=(kc == 0),
                                stop=(kc == KC - 1),
                            )
                            if kc % cfg["ygran"] == cfg["ygran"] - 1:
                                yield
                        for t in rope_thunks(m, n, ps):
                            t()
                            yield

                # ---- A0: v projection + k heads + q head 0 (pure PE phase) ----
                with tc.tile_pool(name="wvp", bufs=1) as wvp:
                    wv_sb = wvp.tile([128, KC, HKV * HD], BF16, name="wv")
                    wvsrc = wv3.rearrange("p (kc c) -> p kc c", kc=KC)
                    xTr = xT.rearrange("(kc p) t -> p kc t", p=128)
                    # all loads issued up front, ordered by first use so the
                    # DMA engine streams while the PE consumes: x(strip 0)
                    # per-kc with wv/wk/wq interleaved at their first-need
                    # points, trig per strip, strip 1 per-kc (sems fire
                    # progressively), strips 2-3 as single big copies, mask
                    # last.
                    def dma_x(kc, n):
                        nc.sync.dma_start(
                            xs[:, kc, bass.ts(n, 512)], xTr[:, kc, bass.ts(n, 512)]
                        )

                    def dma_wv(i):
                        nc.sync.dma_start(
                            wv_sb[:, 4 * i : 4 * i + 4, :],
                            wvsrc[:, 4 * i : 4 * i + 4, :],
                        )

                    def dma_trig(n):
                        nc.sync.dma_start(
                            trigf_sb[:, bass.ts(n, 512)], trigf[:, bass.ts(n, 512)]
                        )
                        nc.sync.dma_start(
                            trigw_sb[:, bass.ts(n, 512)], trigw[:, bass.ts(n, 512)]
                        )

                    dma_x(0, 0)
                    dma_wv(0)
                    wk0 = load_wm(HQ)
                    dma_x(1, 0)
                    dma_x(2, 0)
                    wk1 = load_wm(HQ + 1)
                    dma_x(3, 0)
                    dma_wv(1)
                    wq0 = load_wm(0)
                    for kc in range(4, 8):
                        dma_x(kc, 0)
                    dma_wv(2)
                    for kc in range(8, 12):
                        dma_x(kc, 0)
                    dma_wv(3)
                    for kc in range(12, KC):
                        dma_x(kc, 0)
                    dma_trig(0)
                    for kc in range(KC):
                        dma_x(kc, 1)
                    dma_trig(1)
                    nc.sync.dma_start(
                        xs[:, :, bass.ts(2, 512)], xTr[:, :, bass.ts(2, 512)]
                    )
                    dma_trig(2)
                    nc.sync.dma_start(
                        xs[:, :, bass.ts(3, 512)], xTr[:, :, bass.ts(3, 512)]
                    )
                    dma_trig(3)
                    nc.sync.dma_start(mask_sb[:], maskd.rearrange("d p q -> p d q"))

                    # per-kc interleave: the PE tracks the x DMA stream (one
                    # kc's worth of v+k+q matmuls per arriving tile) instead
                    # of waiting for a full strip.  k/q matmuls lag the v
                    # matmuls by 2 kc so their weight DMAs have landed.
                    LAG = cfg.get("lag", 2)
                    for n in range(NQ):
                        nsl = bass.ts(n, 512)
                        if tag:
                            tag(nc, f"A0 n{n}")
                        kq = [
                            (HQ, wk0, psY.tile([128, 512], F32, name="psy")),
                            (HQ + 1, wk1, psY.tile([128, 512], F32, name="psy")),
                            (0, wq0, psA.tile([128, 512], F32, name="psA")),
                        ]
                        # two half-passes of 2 token-tiles each, one PSUM
                        # tile per token-tile (independent accumulation
                        # groups must not share a tile); k0/k1 lag the pass-0
                        # v matmuls by LAG kc, q0 rides pass 1, so each
                        # projection starts right as its weight DMA lands
                        # while pass 0 paces the x stream.
                        for pas in range(2):
                            vt = [
                                psS.tile([128, 512], F32, name="pss")
                                for _ in range(2)
                            ]
                            for kcv in range(KC + (LAG if pas == 0 else 0)):
                                if kcv < KC:
                                    for i in range(2):
                                        tt = 4 * n + 2 * pas + i
                                        nc.tensor.matmul(
                                            vt[i][:, 0 : HKV * HD],
                                            xs[:, kcv, tt * 128 : (tt + 1) * 128],
                                            wv_sb[:, kcv, :],
                                            start=(kcv == 0),
                                            stop=(kcv == KC - 1),
                                        )
                                    if pas == 1:
                                        for m, w, ps in kq[2:]:
                                            nc.tensor.matmul(
                                                ps[:],
                                                w[:, kcv, :],
                                                xs[:, kcv, nsl],
                                                start=(kcv == 0),
                                                stop=(kcv == KC - 1),
                                            )
                                if pas == 0:
                                    kc = kcv - LAG
                                    if kc >= 0:
                                        for m, w, ps in kq[:2]:
                                            nc.tensor.matmul(
                                                ps[:],
                                                w[:, kc, :],
                                                xs[:, kc, nsl],
                                                start=(kc == 0),
                                                stop=(kc == KC - 1),
                                            )
                            for i in range(2):
                                tt = 4 * n + 2 * pas + i
                                nc.scalar.copy(
                                    v_sb[:, tt, :], vt[i][:, 0 : HKV * HD]
                                )
                        for m, w, ps in kq:
                            for t in rope_thunks(m, n, ps):
                                t()

                # ---- heads 0..6: attention + next head's projection ----
                # preload the first 4 c_proj weight tiles while the DMA
                # engine is idle (the rest need xs's SBUF, freed after head 6)
                wmca = wm_early.tile([128, 4, HQ, 128], BF16, name="wpcearly")
                for fm in range(4):
                    nc.sync.dma_start(wmca[:, fm, :, :], wp5[fm, :, :, :])
                agens = {}

                def get_agen(hn):
                    if hn not in agens and hn < HQ:
                        agens[hn] = a_stream(hn, psA)
                    return agens.get(hn)

                def pop06(h, qj, kt):
                    g = get_agen(h + 1)
                    if g is None:
                        return
                    next(g, None)
                    pm = cfg["pop_mode"]
                    extra = (
                        (kt < 5 or kt >= 10) if pm == 0
                        else kt >= 4 if pm == 1
                        else True if pm == 2
                        else (kt < 4 * qj) if pm == 3
                        else (kt < 4 * qj and qj < 3) if pm == 5
                        else (kt >= 4 * qj - 4) if pm == 6
                        else (kt >= 4 * qj - 2) if pm == 7
                        else False
                    )
                    if extra:
                        next(g, None)

                def head_end06(h):
                    g = get_agen(h + 1)
                    if g is not None:
                        for _ in g:
                            pass

                for h in range(HQ - 1):
                    for qj in cfg.get("sorder", (0, 1, 2, 3)):
                        ps_y, ssum = emit_strip(h, qj, pop06)
                        if state["pending"] is not None:
                            finalize(*state["pending"])
                        state["pending"] = (h, qj, ps_y, ssum)
                    head_end06(h)

            # ---- head 7: attention + output projection interleaved ----
            with (
                tc.tile_pool(name="wpc", bufs=1) as wpc,
                tc.tile_pool(name="obp", bufs=cfg.get("obp", 3)) as obp,
                tc.tile_pool(name="psO", bufs=2, space="PSUM") as psO,
                tc.tile_pool(name="psX", bufs=1, space="PSUM") as psX,
            ):
                # all 16 c_proj weight tiles resident (loaded once; xs freed
                # the SBUF above); per-fm DMAs so sems fire progressively
                wmc_all = wpc.tile([128, FM - 4, HQ, 128], BF16, name="wpcall")
                for fm in range(4, FM):
                    nc.sync.dma_start(wmc_all[:, fm - 4, :, :], wp5[fm, :, :, :])

                def wmc(fm):
                    return wmca[:, fm, :, :] if fm < 4 else wmc_all[:, fm - 4, :, :]

                def c_stream(n):
                    """Output projection for token strip n (16 feature tiles)."""
                    nsl = bass.ts(n, 512)
                    for fm in range(FM):
                        yield
                        if tag:
                            tag(nc, f"cproj n{n} fm{fm}")
                        last = n == NQ - 1 and fm == FM - 1
                        if last:
                            # final tile: compute/copy/store in column halves
                            # (separate PSUM tiles) so the closing DMA chain
                            # is half as deep
                            for c0 in (0, 256):
                                ps_h = psO.tile([128, 512], F32, name="pso")[:, 0:256]
                                for h2 in range(HQ):
                                    nc.tensor.matmul(
                                        ps_h[:],
                                        wmc(fm)[:, h2, :],
                                        yt[:, h2, n * 512 + c0 : n * 512 + c0 + 256],
                                        start=(h2 == 0),
                                        stop=(h2 == HQ - 1),
                                    )
                                ob = obp.tile([128, 256], F32, name="obh")
                                nc.scalar.copy(ob[:], ps_h[:])
                                nc.sync.dma_start(
                                    outT[
                                        fm * 128 : (fm + 1) * 128,
                                        n * 512 + c0 : n * 512 + c0 + 256,
                                    ],
                                    ob[:],
                                )
                            yield
                            continue
                        ps_o = psO.tile([128, 512], F32, name="pso")
                        for h2 in range(HQ):
                            nc.tensor.matmul(
                                ps_o[:],
                                wmc(fm)[:, h2, :],
                                yt[:, h2, nsl],
                                start=(h2 == 0),
                                stop=(h2 == HQ - 1),
                            )
                            if h2 % 2 == 1:
                                yield
                        ob = obp.tile([128, 512], F32, name="ob")
                        nc.scalar.copy(ob[:], ps_o[:])
                        nc.sync.dma_start(
                            outT[fm * 128 : (fm + 1) * 128, nsl], ob[:]
                        )
                        yield

                cgens = []
                cskip = {"n": 0}

                _end = object()

                def pop7(kt):
                    if cskip["n"] > 0:
                        cskip["n"] -= 1
                        return
                    for _ in range(2):
                        while cgens:
                            if next(cgens[0], _end) is _end:
                                cgens.pop(0)
                                continue
                            break

                for qj in cfg.get("sorder", (0, 1, 2, 3)):
                    ps_y, ssum = emit_strip(
                        HQ - 1, qj, lambda h, q, kt: pop7(kt), pss_x=psX
                    )
                    if state["pending"] is not None:
                        finalize(*state["pending"])
                        state["pending"] = None
                    finalize(HQ - 1, qj, ps_y, ssum)
                    if qj == 0:
                        cskip["n"] = cfg.get("cskip", 0)
                    cgens.append(c_stream(qj))
                for g in cgens:
                    for _ in g:
                        pass

            if dbg:
                nc.sync.dma_start(dbg_q[:], qrot[0][:])
                nc.sync.dma_start(dbg_k[:], krot[0][:])
                nc.sync.dma_start(dbg_v[:], v_sb[:])
                nc.sync.dma_start(dbg_y[:], yt[:])

    nc.compile()
    return nc


def _get_nc():
    global _NC
    if _NC is None:
        _NC = build_nc()
    return _NC


def _prep_inputs(x, w_attn, w_proj):
    """Build the 8 per-core input maps from the full-problem arrays."""
    perm = np.concatenate([np.arange(0, HD, 2), np.arange(1, HD, 2)])

    f = np.arange(64, dtype=np.float64)
    inv = ROPE_THETA ** (-2.0 * f / HD)
    ang = inv[:, None] * np.arange(T, dtype=np.float64)[None, :]
    trigc = np.cos(ang).astype(np.float32)
    trigs = np.sin(ang).astype(np.float32)
    trigf = np.ascontiguousarray(np.concatenate([trigc, trigc], axis=0)).astype(BF)
    trigw = np.ascontiguousarray(np.concatenate([trigs, trigs], axis=0)).astype(BF)

    kk = np.arange(128)[None, :, None]
    qq = np.arange(512)[None, None, :]
    dd = np.arange(4)[:, None, None]
    maskd = ((128 * dd + kk) <= qq).astype(BF)

    w_attn = np.asarray(w_attn)
    w_proj = np.asarray(w_proj)
    x = np.asarray(x)

    in_maps = []
    for core in range(N_CORES):
        b, g = core // TP, core % TP
        xTa = np.ascontiguousarray(x[b].T).astype(BF)

        qrows = []
        for h in range(HQ):
            gh = g * HQ + h
            qrows.append(gh * HD + perm)
        for kv in range(HKV):
            gk = g * HKV + kv
            qrows.append(N_HEAD * HD + gk * HD + perm)
        qrows = np.concatenate(qrows)
        wqk = w_attn[qrows].astype(BF)  # [1280, C]
        # wqk3[m, p, kc*128+col] = wqk[m*128+col, kc*128+p]
        wqk3 = np.ascontiguousarray(
            wqk.reshape(MQK, 128, KC, 128).transpose(0, 3, 2, 1).reshape(MQK, 128, C)
        )

        vrows = np.concatenate(
            [
                (N_HEAD + N_KV_HEAD) * HD + (g * HKV + kv) * HD + np.arange(HD)
                for kv in range(HKV)
            ]
        )
        wv = w_attn[vrows].astype(BF)  # [256, C]
        # wv3[p, kc*256+c] = wv[c, kc*128+p]
        wv3 = np.ascontiguousarray(
            wv.reshape(HKV * HD, KC, 128).transpose(2, 1, 0).reshape(128, KC * HKV * HD)
        )

        cols = np.arange(g * HQ * HD, (g + 1) * HQ * HD)
        wpg = w_proj[:, cols].astype(BF)  # [C, 1024], rows = out features
        # wp5[fm, d, h, p] = wpg[fm*128+p, h*128+d]
        wp5 = np.ascontiguousarray(
            wpg.T.reshape(HQ, 128, FM, 128).transpose(2, 1, 0, 3)
        )

        in_maps.append(
            {
                "xT": xTa,
                "wqk3": wqk3,
                "wv3": wv3,
                "wp5": wp5,
                "trigf": trigf,
                "trigw": trigw,
                "maskd": maskd,
            }
        )
    return in_maps


def kernel(x, w_attn, w_proj):
    global LAST_RUN
    nc = _get_nc()
    in_maps = _prep_inputs(x, w_attn, w_proj)
    res = run_bass_kernel_spmd(nc, in_maps, core_ids=list(range(N_CORES)))
    LAST_RUN = res
    out = np.empty((B, T, C), dtype=np.float32)
    for b in range(B):
        acc = res.results[TP * b]["outT"] + res.results[TP * b + 1]["outT"]
        out[b] = acc.T
    return out



# revision 65
# speedup vs baseline: 1.0005x; 1.0001x over previous
"""Causal self-attention (GQA + RoPE) Trainium2 kernel, 8-way sharded.

Sharding: DP=4 over batch x TP=2 over kv-head groups (2 kv heads + their
8 q heads per group).  Each core computes its batch's qkv projection for
its head group, causal attention, and a partial c_proj (columns of
w_proj for its head group).  Host sums the two partial c_proj outputs
per batch.

Everything on-chip runs transposed ([feature, token] layout) so matmuls
contract along partitions; host transposes inputs/outputs.

Pipeline: the attention inner loop is ACT-bound (one exp per QK tile),
so the q/k projection + RoPE work for head h+1 is interleaved into the
PE stream of head h's attention, keeping the PE busy while ACT churns
through exps.

RoPE: w_attn q/k rows are permuted per-head to [even dims; odd dims] so
rotation pairs land at partition f and f+64 of the qkv psum tile:
  P  = ps * [c; c] (SBUF),  P2 = ps * [s; s] (PSUM)
  out[0:64]   = P[0:64]  - P2[64:128]
  out[64:128] = P2[0:64] + P[64:128]
(each combine reads one SBUF + one PSUM operand, which may sit at
different base partitions; two SBUF operands may not).

Softmax: att^T tiles ([k, q] layout) are exp'd on ACT without
max-subtraction (logits are O(6), fp32-safe).  Denominators: groups of
4 e-tiles are tree-summed on DVE and hit with one ones-column matmul
per group (deferred into the next group's PE stream); the per-q
reciprocal is broadcast down partitions with a f32r outer-product
matmul, also deferred one q-tile.
"""

import math

import numpy as np
import ml_dtypes

import concourse.bass as bass
import concourse.bass_isa as bass_isa
import concourse.mybir as mybir
import concourse.tile as tile
from concourse import bacc
from concourse.bass_utils import run_bass_kernel_spmd

ALU = mybir.AluOpType
AF = mybir.ActivationFunctionType
F32 = mybir.dt.float32
F32R = mybir.dt.float32r
BF16 = mybir.dt.bfloat16
BF = ml_dtypes.bfloat16

# problem shape (hardcoded per contest rules)
B, T, C = 4, 2048, 2048
N_HEAD, N_KV_HEAD, HD = 16, 4, 128
ROPE_THETA = 10000.0

TP = 2            # head-group shards
DP = 4            # batch shards
HQ = N_HEAD // TP         # 8 q heads per core
HKV = N_KV_HEAD // TP     # 2 kv heads per core
NREP = N_HEAD // N_KV_HEAD  # 4
QK_ROWS = (HQ + HKV) * HD   # 1280
KC = C // 128     # 16 contraction tiles
NQ = T // 512     # 4 token strips
MQK = QK_ROWS // 128  # 10 feature tiles (8 q heads + 2 kv heads)
FM = C // 128     # 16 output feature tiles
SCALE = 1.0 / math.sqrt(HD)

N_CORES = 8

_NC = None        # cached compiled Bass module
LAST_RUN = None   # BassKernelResults of the most recent kernel() call


def build_nc(dbg=False, tag=None, cfg=None):
    cfg = {**dict(look=1, pop_mode=7, ygran=2, eb=10), **(cfg or {})}
    nc = bacc.Bacc(None, target_bir_lowering=False, debug=False)

    xT = nc.declare_dram_parameter("xT", [C, T], BF16, isOutput=False)
    wqk3 = nc.declare_dram_parameter("wqk3", [MQK, 128, C], BF16, isOutput=False)
    wv3 = nc.declare_dram_parameter("wv3", [128, KC * HKV * HD], BF16, isOutput=False)
    wp5 = nc.declare_dram_parameter("wp5", [FM, 128, HQ, 128], BF16, isOutput=False)
    trigf = nc.declare_dram_parameter("trigf", [128, T], BF16, isOutput=False)  # [c;c]
    trigw = nc.declare_dram_parameter("trigw", [128, T], BF16, isOutput=False)  # [s;s]
    maskd = nc.declare_dram_parameter("maskd", [4, 128, 512], BF16, isOutput=False)
    outT = nc.declare_dram_parameter("outT", [C, T], F32, isOutput=True)
    if dbg:
        dbg_q = nc.declare_dram_parameter("dbg_q", [128, T], BF16, isOutput=True)
        dbg_k = nc.declare_dram_parameter("dbg_k", [128, T], BF16, isOutput=True)
        dbg_v = nc.declare_dram_parameter(
            "dbg_v", [128, T // 128, HKV * HD], BF16, isOutput=True
        )
        dbg_y = nc.declare_dram_parameter("dbg_y", [128, HQ, T], BF16, isOutput=True)

    with tile.TileContext(nc) as tc:
        with (
            tc.tile_pool(name="const", bufs=1) as const,
            tc.tile_pool(name="persist", bufs=1) as persist,
            tc.tile_pool(name="eb", bufs=cfg.get("eb", 8)) as eb,
            tc.tile_pool(name="gag", bufs=cfg.get("gag", 2)) as gag,
            tc.tile_pool(name="smp", bufs=cfg.get("smp", 2)) as smp,
            tc.tile_pool(name="srp", bufs=cfg.get("srp", 2)) as srp,
            tc.tile_pool(name="wmear", bufs=1) as wm_early,
            tc.tile_pool(name="psS", bufs=3, space="PSUM") as psS,
            tc.tile_pool(name="psY", bufs=2, space="PSUM") as psY,
        ):
            trigf_sb = const.tile([128, T], BF16, name="trigf")
            trigw_sb = const.tile([128, T], BF16, name="trigw")
            mask_sb = const.tile([128, 4, 512], BF16, name="mask")

            qrot = [persist.tile([128, T], BF16, name=f"qrot{h}") for h in range(HQ)]
            krot = [persist.tile([128, T], BF16, name=f"krot{h}") for h in range(HKV)]
            v_sb = persist.tile([128, T // 128, HKV * HD], BF16, name="vtok")
            yt = persist.tile([128, HQ, T], BF16, name="yt")

            state = {"pending": None}

            def finalize(h, qj, ps_y, ssum):
                if tag:
                    tag(nc, f"finalize h{h} qj{qj}")
                nc.vector.reciprocal(ssum[:], ssum[:])
                nc.vector.tensor_tensor(
                    yt[:, h, bass.ts(qj, 512)], ps_y[:], ssum[:], ALU.mult
                )

            def stage_a(h, qj, kt, pss_x=None):
                """QK matmul + exp (+ causal mask for diagonal tiles)."""
                kvh = h // NREP
                d = kt - 4 * qj
                lo = 128 * d if d > 0 else 0
                qlo = qj * 512 + lo
                if pss_x is not None and kt % 4 == 3:
                    ps_s = pss_x.tile([128, 512], F32, name="pssx")
                else:
                    ps_s = psS.tile([128, 512], F32, name="pss")
                if tag:
                    tag(nc, f"QK h{h} qj{qj} kt{kt}")
                nc.tensor.matmul(
                    ps_s[:, lo:512],
                    krot[kvh][:, kt * 128 : (kt + 1) * 128],
                    qrot[h][:, qlo : (qj + 1) * 512],
                    start=True,
                    stop=True,
                )
                e = eb.tile([128, 512], BF16, name="e")
                nc.scalar.activation(
                    e[:, lo:512], ps_s[:, lo:512], AF.Exp, scale=SCALE
                )
                if d >= 0:
                    nc.vector.tensor_tensor(
                        e[:, lo:512], e[:, lo:512],
                        mask_sb[:, d, lo:512], ALU.mult,
                    )
                return e

            def stage_b(c, h, qj, kt, e):
                """AV matmul + strip-sum accumulation for tile kt.

                On the strip's last tile, issues the GPSIMD
                partition_all_reduce and returns the ssum tile."""
                kvh = h // NREP
                nkt = 4 * qj + 4
                d = kt - 4 * qj
                lo = 128 * d if d > 0 else 0
                if kt == 0:
                    c["ps_y"] = psY.tile([128, 512], F32, name="psy")
                s = c["s"]
                tree = c["tree"]
                if tag:
                    tag(nc, f"AV h{h} qj{qj} kt{kt}")
                nc.tensor.matmul(
                    c["ps_y"][:, lo:512],
                    v_sb[:, kt, kvh * HD : (kvh + 1) * HD],
                    e[:, lo:512],
                    start=(kt == 0),
                    stop=(kt == nkt - 1),
                )
                if d >= 0:
                    if s is None:
                        # qj == 0, d == 0: seed the strip sum
                        s = smp.tile([128, 512], BF16, name="s")
                        c["s"] = s
                        nc.vector.tensor_copy(s[:], e[:])
                    else:
                        nc.vector.tensor_tensor(
                            s[:, lo:512], s[:, lo:512], e[:, lo:512], ALU.add
                        )
                else:
                    # full groups: tree-sum 4 e-tiles on DVE, then merge
                    # into the strip sum.
                    ph = kt % 4
                    if ph == 0:
                        tree["g0"] = e
                    elif ph == 1:
                        ga = gag.tile([128, 512], BF16, name="ga")
                        nc.vector.tensor_tensor(ga[:], tree["g0"][:], e[:], ALU.add)
                        tree["ga"] = ga
                    elif ph == 2:
                        tree["g2"] = e
                    else:
                        if s is None:
                            s = smp.tile([128, 512], BF16, name="s")
                            c["s"] = s
                            gs = s
                        else:
                            gs = gag.tile([128, 512], BF16, name="gs")
                        nc.vector.tensor_tensor(gs[:], tree["g2"][:], e[:], ALU.add)
                        nc.vector.tensor_tensor(gs[:], gs[:], tree["ga"][:], ALU.add)
                        if gs is not s:
                            nc.vector.tensor_tensor(s[:], s[:], gs[:], ALU.add)
                if kt == nkt - 1:
                    ssum = srp.tile([128, 512], F32, name="ssum")
                    nc.gpsimd.partition_all_reduce(
                        ssum[:], s[:], 128, bass_isa.ReduceOp.add
                    )
                    return ssum
                return None

            def emit_strip(h, qj, pop, pss_x=None):
                """One attention strip, QK/exp one tile ahead of AV so the
                AV matmul never waits on ACT's exp latency."""
                c = {"s": None, "tree": {}, "ps_y": None}
                nkt = 4 * qj + 4
                e_prev = stage_a(h, qj, 0, pss_x)
                for kt in range(1, nkt):
                    e_cur = stage_a(h, qj, kt, pss_x)
                    pop(h, qj, kt - 1)
                    stage_b(c, h, qj, kt - 1, e_prev)
                    e_prev = e_cur
                ssum = stage_b(c, h, qj, nkt - 1, e_prev)
                pop(h, qj, nkt - 1)
                return c["ps_y"], ssum

            # ======== projection machinery (lives through heads 0..6) ========
            with (
                tc.tile_pool(name="xa", bufs=1) as xa,
                tc.tile_pool(name="wm", bufs=cfg.get("wm", 3)) as wm,
                tc.tile_pool(name="ta", bufs=1) as ta,
                tc.tile_pool(name="psA", bufs=2, space="PSUM") as psA,
                tc.tile_pool(name="psP2", bufs=1, space="PSUM") as psP2,
            ):
                xs = xa.tile([128, KC, T], BF16, name="xs")

                def load_wm(m, split=1):
                    w = wm.tile([128, KC, 128], BF16, name="wm")
                    wsrc = wqk3[m, :, :].rearrange("p (kc c) -> p kc c", kc=KC)
                    step = KC // split
                    chunks = []
                    for i in range(split):
                        chunks.append(
                            lambda i=i: nc.sync.dma_start(
                                w[:, i * step : (i + 1) * step, :],
                                wsrc[:, i * step : (i + 1) * step, :],
                            )
                        )
                    if split == 1:
                        chunks[0]()
                        return w
                    return w, chunks

                def rope_thunks(m, n, ps):
                    """The four RoPE ops for one (feature tile, strip) pair,
                    as emission thunks (must be called in list order).  The
                    sine product goes to a PSUM scratch tile so `ps` (the
                    projection accumulator) is released after the two
                    products, and so each combine reads one SBUF + one PSUM
                    operand at different base partitions."""
                    dst = qrot[m] if m < HQ else krot[m - HQ]
                    nsl = bass.ts(n, 512)
                    box = {}

                    def t0():
                        if tag:
                            tag(nc, f"rope m{m} n{n}")
                        box["pt"] = ta.tile([128, 512], F32, name="pt")
                        nc.vector.tensor_tensor(
                            box["pt"][:], ps[:], trigf_sb[:, nsl], ALU.mult
                        )

                    def t1():
                        box["p2"] = psP2.tile([128, 512], F32, name="p2")
                        nc.vector.tensor_tensor(
                            box["p2"][:], ps[:], trigw_sb[:, nsl], ALU.mult
                        )

                    def t2():
                        nc.vector.tensor_tensor(
                            dst[0:64, nsl], box["pt"][0:64, :],
                            box["p2"][64:128, :], ALU.subtract,
                        )

                    def t3():
                        nc.vector.tensor_tensor(
                            dst[64:128, nsl], box["p2"][0:64, :],
                            box["pt"][64:128, :], ALU.add,
                        )

                    return [t0, t1, t2, t3]

                def a_stream(m, pool):
                    if tag:
                        tag(nc, f"a_stream m{m} load_wm")
                    w = load_wm(m)
                    yield
                    for n in range(NQ):
                        nsl = bass.ts(n, 512)
                        if tag:
                            tag(nc, f"a_stream m{m} n{n} mm")
                        ps = pool.tile([128, 512], F32, name="psA")
                        for kc in range(KC):
                            nc.tensor.matmul(
                                ps[:],
                                w[:, kc, :],
                                xs[:, kc, nsl],
                                start=(kc == 0),
                                stop=(kc == KC - 1),
                            )
                            if kc % cfg["ygran"] == cfg["ygran"] - 1:
                                yield
                        for t in rope_thunks(m, n, ps):
                            t()
                            yield

                # ---- A0: v projection + k heads + q head 0 (pure PE phase) ----
                with tc.tile_pool(name="wvp", bufs=1) as wvp:
                    wv_sb = wvp.tile([128, KC, HKV * HD], BF16, name="wv")
                    wvsrc = wv3.rearrange("p (kc c) -> p kc c", kc=KC)
                    xTr = xT.rearrange("(kc p) t -> p kc t", p=128)
                    # all loads issued up front, ordered by first use so the
                    # DMA engine streams while the PE consumes: x(strip 0)
                    # per-kc with wv/wk/wq interleaved at their first-need
                    # points, trig per strip, strip 1 per-kc (sems fire
                    # progressively), strips 2-3 as single big copies, mask
                    # last.
                    def dma_x(kc, n):
                        nc.sync.dma_start(
                            xs[:, kc, bass.ts(n, 512)], xTr[:, kc, bass.ts(n, 512)]
                        )

                    def dma_wv(i):
                        nc.sync.dma_start(
                            wv_sb[:, 4 * i : 4 * i + 4, :],
                            wvsrc[:, 4 * i : 4 * i + 4, :],
                        )

                    def dma_trig(n):
                        nc.sync.dma_start(
                            trigf_sb[:, bass.ts(n, 512)], trigf[:, bass.ts(n, 512)]
                        )
                        nc.sync.dma_start(
                            trigw_sb[:, bass.ts(n, 512)], trigw[:, bass.ts(n, 512)]
                        )

                    dma_x(0, 0)
                    dma_wv(0)
                    wk0 = load_wm(HQ)
                    dma_x(1, 0)
                    dma_x(2, 0)
                    wk1 = load_wm(HQ + 1)
                    dma_x(3, 0)
                    dma_wv(1)
                    wq0 = load_wm(0)
                    for kc in range(4, 8):
                        dma_x(kc, 0)
                    dma_wv(2)
                    for kc in range(8, 12):
                        dma_x(kc, 0)
                    dma_wv(3)
                    for kc in range(12, KC):
                        dma_x(kc, 0)
                    dma_trig(0)
                    for kc in range(KC):
                        dma_x(kc, 1)
                    dma_trig(1)
                    nc.sync.dma_start(
                        xs[:, :, bass.ts(2, 512)], xTr[:, :, bass.ts(2, 512)]
                    )
                    dma_trig(2)
                    nc.sync.dma_start(
                        xs[:, :, bass.ts(3, 512)], xTr[:, :, bass.ts(3, 512)]
                    )
                    dma_trig(3)
                    nc.sync.dma_start(mask_sb[:], maskd.rearrange("d p q -> p d q"))

                    # per-kc interleave: the PE tracks the x DMA stream (one
                    # kc's worth of v+k+q matmuls per arriving tile) instead
                    # of waiting for a full strip.  k/q matmuls lag the v
                    # matmuls by 2 kc so their weight DMAs have landed.
                    LAG = cfg.get("lag", 2)
                    for n in range(NQ):
                        nsl = bass.ts(n, 512)
                        if tag:
                            tag(nc, f"A0 n{n}")
                        kq = [
                            (HQ, wk0, psY.tile([128, 512], F32, name="psy")),
                            (HQ + 1, wk1, psY.tile([128, 512], F32, name="psy")),
                            (0, wq0, psA.tile([128, 512], F32, name="psA")),
                        ]
                        # two half-passes of 2 token-tiles each, one PSUM
                        # tile per token-tile (independent accumulation
                        # groups must not share a tile); k0/k1 lag the pass-0
                        # v matmuls by LAG kc, q0 rides pass 1, so each
                        # projection starts right as its weight DMA lands
                        # while pass 0 paces the x stream.
                        for pas in range(2):
                            vt = [
                                psS.tile([128, 512], F32, name="pss")
                                for _ in range(2)
                            ]
                            for kcv in range(KC + (LAG if pas == 0 else 0)):
                                if kcv < KC:
                                    for i in range(2):
                                        tt = 4 * n + 2 * pas + i
                                        nc.tensor.matmul(
                                            vt[i][:, 0 : HKV * HD],
                                            xs[:, kcv, tt * 128 : (tt + 1) * 128],
                                            wv_sb[:, kcv, :],
                                            start=(kcv == 0),
                                            stop=(kcv == KC - 1),
                                        )
                                    if pas == 1:
                                        for m, w, ps in kq[2:]:
                                            nc.tensor.matmul(
                                                ps[:],
                                                w[:, kcv, :],
                                                xs[:, kcv, nsl],
                                                start=(kcv == 0),
                                                stop=(kcv == KC - 1),
                                            )
                                if pas == 0:
                                    kc = kcv - LAG
                                    if kc >= 0:
                                        for m, w, ps in kq[:2]:
                                            nc.tensor.matmul(
                                                ps[:],
                                                w[:, kc, :],
                                                xs[:, kc, nsl],
                                                start=(kc == 0),
                                                stop=(kc == KC - 1),
                                            )
                            for i in range(2):
                                tt = 4 * n + 2 * pas + i
                                nc.scalar.copy(
                                    v_sb[:, tt, :], vt[i][:, 0 : HKV * HD]
                                )
                        for m, w, ps in kq:
                            for t in rope_thunks(m, n, ps):
                                t()

                # ---- heads 0..6: attention + next head's projection ----
                # preload the first 4 c_proj weight tiles while the DMA
                # engine is idle (the rest need xs's SBUF, freed after head 6)
                wmca = wm_early.tile([128, 4, HQ, 128], BF16, name="wpcearly")
                for fm in range(4):
                    nc.sync.dma_start(wmca[:, fm, :, :], wp5[fm, :, :, :])
                agens = {}

                def get_agen(hn):
                    if hn not in agens and hn < HQ:
                        agens[hn] = a_stream(hn, psA)
                    return agens.get(hn)

                def pop06(h, qj, kt):
                    g = get_agen(h + 1)
                    if g is None:
                        return
                    next(g, None)
                    pm = cfg["pop_mode"]
                    extra = (
                        (kt < 5 or kt >= 10) if pm == 0
                        else kt >= 4 if pm == 1
                        else True if pm == 2
                        else (kt < 4 * qj) if pm == 3
                        else (kt < 4 * qj and qj < 3) if pm == 5
                        else (kt >= 4 * qj - 4) if pm == 6
                        else (kt >= 4 * qj - 2) if pm == 7
                        else False
                    )
                    if extra:
                        next(g, None)

                def head_end06(h):
                    g = get_agen(h + 1)
                    if g is not None:
                        for _ in g:
                            pass

                for h in range(HQ - 1):
                    for qj in cfg.get("sorder", (0, 1, 2, 3)):
                        ps_y, ssum = emit_strip(h, qj, pop06)
                        if state["pending"] is not None:
                            finalize(*state["pending"])
                        state["pending"] = (h, qj, ps_y, ssum)
                    head_end06(h)

            # ---- head 7: attention + output projection interleaved ----
            with (
                tc.tile_pool(name="wpc", bufs=1) as wpc,
                tc.tile_pool(name="obp", bufs=cfg.get("obp", 3)) as obp,
                tc.tile_pool(name="psO", bufs=2, space="PSUM") as psO,
                tc.tile_pool(name="psX", bufs=1, space="PSUM") as psX,
            ):
                # all 16 c_proj weight tiles resident (loaded once; xs freed
                # the SBUF above); per-fm DMAs so sems fire progressively
                wmc_all = wpc.tile([128, FM - 4, HQ, 128], BF16, name="wpcall")
                for fm in range(4, FM):
                    nc.sync.dma_start(wmc_all[:, fm - 4, :, :], wp5[fm, :, :, :])

                def wmc(fm):
                    return wmca[:, fm, :, :] if fm < 4 else wmc_all[:, fm - 4, :, :]

                def c_stream(n):
                    """Output projection for token strip n (16 feature tiles)."""
                    nsl = bass.ts(n, 512)
                    for fm in range(FM):
                        yield
                        if tag:
                            tag(nc, f"cproj n{n} fm{fm}")
                        last = n == NQ - 1 and fm == FM - 1
                        if last:
                            # final tile: compute/copy/store in column halves
                            # (separate PSUM tiles) so the closing DMA chain
                            # is half as deep
                            for c0 in (0, 256):
                                ps_h = psO.tile([128, 512], F32, name="pso")[:, 0:256]
                                for h2 in range(HQ):
                                    nc.tensor.matmul(
                                        ps_h[:],
                                        wmc(fm)[:, h2, :],
                                        yt[:, h2, n * 512 + c0 : n * 512 + c0 + 256],
                                        start=(h2 == 0),
                                        stop=(h2 == HQ - 1),
                                    )
                                ob = obp.tile([128, 256], F32, name="obh")
                                nc.scalar.copy(ob[:], ps_h[:])
                                nc.sync.dma_start(
                                    outT[
                                        fm * 128 : (fm + 1) * 128,
                                        n * 512 + c0 : n * 512 + c0 + 256,
                                    ],
                                    ob[:],
                                )
                            yield
                            continue
                        ps_o = psO.tile([128, 512], F32, name="pso")
                        for h2 in range(HQ):
                            nc.tensor.matmul(
                                ps_o[:],
                                wmc(fm)[:, h2, :],
                                yt[:, h2, nsl],
                                start=(h2 == 0),
                                stop=(h2 == HQ - 1),
                            )
                            if h2 % 2 == 1:
                                yield
                        ob = obp.tile([128, 512], F32, name="ob")
                        nc.scalar.copy(ob[:], ps_o[:])
                        nc.sync.dma_start(
                            outT[fm * 128 : (fm + 1) * 128, nsl], ob[:]
                        )
                        yield

                cgens = []
                cskip = {"n": 0}

                _end = object()

                def pop7(kt):
                    if cskip["n"] > 0:
                        cskip["n"] -= 1
                        return
                    for _ in range(2):
                        while cgens:
                            if next(cgens[0], _end) is _end:
                                cgens.pop(0)
                                continue
                            break

                for qj in cfg.get("sorder", (0, 1, 2, 3)):
                    ps_y, ssum = emit_strip(
                        HQ - 1, qj, lambda h, q, kt: pop7(kt), pss_x=psX
                    )
                    if state["pending"] is not None:
                        finalize(*state["pending"])
                        state["pending"] = None
                    finalize(HQ - 1, qj, ps_y, ssum)
                    if qj == 0:
                        cskip["n"] = cfg.get("cskip", 0)
                    cgens.append(c_stream(qj))
                for g in cgens:
                    for _ in g:
                        pass

            if dbg:
                nc.sync.dma_start(dbg_q[:], qrot[0][:])
                nc.sync.dma_start(dbg_k[:], krot[0][:])
                nc.sync.dma_start(dbg_v[:], v_sb[:])
                nc.sync.dma_start(dbg_y[:], yt[:])

    nc.compile()
    return nc


def _get_nc():
    global _NC
    if _NC is None:
        _NC = build_nc()
    return _NC


def _prep_inputs(x, w_attn, w_proj):
    """Build the 8 per-core input maps from the full-problem arrays."""
    perm = np.concatenate([np.arange(0, HD, 2), np.arange(1, HD, 2)])

    f = np.arange(64, dtype=np.float64)
    inv = ROPE_THETA ** (-2.0 * f / HD)
    ang = inv[:, None] * np.arange(T, dtype=np.float64)[None, :]
    trigc = np.cos(ang).astype(np.float32)
    trigs = np.sin(ang).astype(np.float32)
    trigf = np.ascontiguousarray(np.concatenate([trigc, trigc], axis=0)).astype(BF)
    trigw = np.ascontiguousarray(np.concatenate([trigs, trigs], axis=0)).astype(BF)

    kk = np.arange(128)[None, :, None]
    qq = np.arange(512)[None, None, :]
    dd = np.arange(4)[:, None, None]
    maskd = ((128 * dd + kk) <= qq).astype(BF)

    w_attn = np.asarray(w_attn)
    w_proj = np.asarray(w_proj)
    x = np.asarray(x)

    in_maps = []
    for core in range(N_CORES):
        b, g = core // TP, core % TP
        xTa = np.ascontiguousarray(x[b].T).astype(BF)

        qrows = []
        for h in range(HQ):
            gh = g * HQ + h
            qrows.append(gh * HD + perm)
        for kv in range(HKV):
            gk = g * HKV + kv
            qrows.append(N_HEAD * HD + gk * HD + perm)
        qrows = np.concatenate(qrows)
        wqk = w_attn[qrows].astype(BF)  # [1280, C]
        # wqk3[m, p, kc*128+col] = wqk[m*128+col, kc*128+p]
        wqk3 = np.ascontiguousarray(
            wqk.reshape(MQK, 128, KC, 128).transpose(0, 3, 2, 1).reshape(MQK, 128, C)
        )

        vrows = np.concatenate(
            [
                (N_HEAD + N_KV_HEAD) * HD + (g * HKV + kv) * HD + np.arange(HD)
                for kv in range(HKV)
            ]
        )
        wv = w_attn[vrows].astype(BF)  # [256, C]
        # wv3[p, kc*256+c] = wv[c, kc*128+p]
        wv3 = np.ascontiguousarray(
            wv.reshape(HKV * HD, KC, 128).transpose(2, 1, 0).reshape(128, KC * HKV * HD)
        )

        cols = np.arange(g * HQ * HD, (g + 1) * HQ * HD)
        wpg = w_proj[:, cols].astype(BF)  # [C, 1024], rows = out features
        # wp5[fm, d, h, p] = wpg[fm*128+p, h*128+d]
        wp5 = np.ascontiguousarray(
            wpg.T.reshape(HQ, 128, FM, 128).transpose(2, 1, 0, 3)
        )

        in_maps.append(
            {
                "xT": xTa,
                "wqk3": wqk3,
                "wv3": wv3,
                "wp5": wp5,
                "trigf": trigf,
                "trigw": trigw,
                "maskd": maskd,
            }
        )
    return in_maps


def kernel(x, w_attn, w_proj):
    global LAST_RUN
    nc = _get_nc()
    in_maps = _prep_inputs(x, w_attn, w_proj)
    res = run_bass_kernel_spmd(nc, in_maps, core_ids=list(range(N_CORES)))
    LAST_RUN = res
    out = np.empty((B, T, C), dtype=np.float32)
    for b in range(B):
        acc = res.results[TP * b]["outT"] + res.results[TP * b + 1]["outT"]
        out[b] = acc.T
    return out

